# revision 19
# baseline (speedup 1.0000x reference)
"""Trainium2 Bass kernel for nn_CACLayer (retrieval + softmax readout + CE).

Computation (see reference):
  att = (q @ db.T) / sqrt(D); w = softmax(att, -1); z = w @ db
  logits = z @ fc_w.T + fc_b; nll = -log_softmax(logits)[targets]; out = mean(nll)

Strategy: data-parallel over batch B=2048 across 8 cores (256 queries each).
Each core streams the full db (both layouts, fp8e4) and fc_w, computes its 256
NLL values on-device; host averages.  All matmuls run in fp8e4 DoubleRow mode
(2 k-tiles of 128 contracted per pass).  Softmax is computed un-normalized
with a -2 bias inside the exp so weights fit e4m3 range (max att*tau ~ 5.7 =>
w <= e^3.7 ~ 40 < 240); the bias cancels in the normalization.  The softmax
sums are accumulated on the (otherwise idle) vector engine during phase A and
partition-reduced with one tiny fp32 matmul, keeping the PE on att/z work.
z is quantized to fp8 with scale S_Z and fc_w with S_W; the combined S_Z*S_W
scale is removed inside the CE exp.  The target logit uses a separate
bf16/f32 path (host-gathered fc_w rows), so no index ops on device.
"""

import os
import sys

for _p in ("/opt/trn_rl_repo", "/root/.axon_site/_ro/trn_rl_repo"):
    if os.path.isdir(_p) and _p not in sys.path:
        sys.path.insert(0, _p)

import numpy as np
import ml_dtypes

import concourse.bass as bass
from concourse import bacc, mybir, tile
from concourse.bass_utils import run_bass_kernel_spmd
from concourse.masks import make_identity

BF16 = mybir.dt.bfloat16
FP8 = mybir.dt.float8e4
F32 = mybir.dt.float32
AF = mybir.ActivationFunctionType
ALU = mybir.AluOpType
AX = mybir.AxisListType
DR = mybir.MatmulPerfMode.DoubleRow

D = 512          # embed dim
N_DB = 32768     # database rows
B = 2048         # batch
C = 10000        # classes
N_CORES = 8
NQ = B // N_CORES          # queries per core (256)
QT = NQ // 128             # q tiles per core (2)
DS = D // 128              # d slices (4)
NST = N_DB // 512          # supertiles of 4 n-tiles (64)
TAU = float(D) ** -0.5
EXP_BIAS = -2.0            # softmax shift; cancels in normalization
S_Z = 128.0                # fp8 scale for z
S_W = 32.0                 # fp8 scale for fc_w

NP_FP8 = ml_dtypes.float8_e4m3   # TRN fp8e4: max 240, IEEE-style inf

_CACHE = {}


def _chunks(total, size):
    out = []
    c0 = 0
    while c0 < total:
        out.append((c0, min(size, total - c0)))
        c0 += size
    return out


def build_nc(nst=NST, n_classes=C, nq=NQ, repeat=1, stream_bufs=6, wexp_bufs=4,
             no_ce=False, no_s=False, no_dma=False, ce_nobias=False,
             ce_noexp=False, s_dve=True, tail_early=True):
    """Build the Bass module.  Parameterized so a scaled-down version can be
    simulated; hardware uses the defaults."""
    qt = nq // 128
    cch = _chunks(n_classes, 512)

    nc = bacc.Bacc("TRN2", target_bir_lowering=False, debug=False)

    qT_d = nc.dram_tensor("qT", [128, DS, nq], FP8, kind="ExternalInput")
    dbT_d = nc.dram_tensor("dbT", [nst, 128, 4, DS, 128], FP8, kind="ExternalInput")
    db_d = nc.dram_tensor("db", [nst, 128, 4, D], FP8, kind="ExternalInput")
    fcw_d = nc.dram_tensor("fcw", [128, DS, n_classes], FP8, kind="ExternalInput")
    fcb_d = nc.dram_tensor("fcb", [1, n_classes], BF16, kind="ExternalInput")
    wt_d = nc.dram_tensor("wt", [128, qt, D], F32, kind="ExternalInput")
    bt_d = nc.dram_tensor("bt", [128, qt], F32, kind="ExternalInput")
    out_d = nc.dram_tensor("nll", [128, qt], F32, kind="ExternalOutput")

    with tile.TileContext(nc) as tc:
        with (
            tc.tile_pool(name="const", bufs=1) as cpool,
            tc.tile_pool(name="stream", bufs=stream_bufs) as spool,
            tc.tile_pool(name="wexp", bufs=wexp_bufs) as wpool,
            tc.tile_pool(name="psA", bufs=2, space="PSUM") as psA,
            tc.tile_pool(name="psAcc", bufs=1, space="PSUM") as psAcc,
        ):
            # ---- resident tensors ----
            qT_sb = cpool.tile([128, DS, nq], FP8)
            nc.sync.dma_start(qT_sb[:], qT_d[:])
            fcw_sb = cpool.tile([128, DS, n_classes], FP8)
            for ds in range(DS):
                nc.sync.dma_start(fcw_sb[:, ds], fcw_d[:, ds])
            fcb_sb = cpool.tile([1, n_classes], BF16)
            nc.sync.dma_start(fcb_sb[:], fcb_d[:])
            wt_sb = cpool.tile([128, qt, D], F32)
            nc.sync.dma_start(wt_sb[:], wt_d[:])
            bt_sb = cpool.tile([128, qt], F32)
            nc.sync.dma_start(bt_sb[:], bt_d[:])

            # 2 k-tiles x 2 columns of ones; padded so the k-tile stride is
            # 16 B (DoubleRow ldweights requires step%16==0).  Both output
            # rows compute the same sum.  (Used when s_dve=False.)
            ones2_f8 = cpool.tile([128, 2, 16], FP8)
            nc.vector.memset(ones2_f8[:], 1.0)
            ones128_f32 = cpool.tile([128, 1], F32)
            nc.vector.memset(ones128_f32[:], 1.0)
            ones1_bf = cpool.tile([1, 128], BF16)
            nc.vector.memset(ones1_bf[:], 1.0)
            ones1_f32 = cpool.tile([1, 128], F32)
            nc.vector.memset(ones1_f32[:], 1.0)
            sz_f32 = cpool.tile([1, 128], F32)
            nc.vector.memset(sz_f32[:], S_Z)
            ebias_sb = cpool.tile([128, 1], F32)
            nc.vector.memset(ebias_sb[:], EXP_BIAS)
            ident = cpool.tile([128, 128], BF16)
            make_identity(nc, ident[:])

            # ---- phase A: att -> exp -> z accumulation over db ----
            # zT_ps [128(d_in), DS, nq] : bank A = ds 0,1 ; bank B = ds 2,3
            zT_ps = psAcc.tile([128, DS, nq], F32)
            # s_ps [2, nq] : unnormalized softmax sums (2 identical rows;
            # only used when s_dve=False)
            s_ps = psAcc.tile([2, nq], F32)

            import contextlib
            rep_cm = tc.For_i(0, repeat, 1) if repeat > 1 else contextlib.nullcontext()
            with rep_cm:
                # 2-stage software pipeline over supertiles: while the ACT
                # engine computes exp(st), the PE runs att matmuls of st+1,
                # so the PE never stalls on the exp dependency.
                if s_dve and not no_s:
                    s_acc = cpool.tile([128, nq], F32)
                    nc.vector.memset(s_acc[:], 0.0)

                db_tiles = {}

                def _load(st):
                    if no_dma and st > 0:
                        db_tiles[st] = db_tiles[0]
                        return
                    dbT_sb = spool.tile([128, 4, DS, 128], FP8, tag="dbT")
                    db_sb = spool.tile([128, 4, D], FP8, tag="db")
                    nc.sync.dma_start(dbT_sb[:], dbT_d[st])
                    nc.sync.dma_start(db_sb[:], db_d[st])
                    db_tiles[st] = (dbT_sb, db_sb)

                att_tiles = {}

                def _att(st):
                    dbT_sb = db_tiles[st][0]
                    att_ps = psA.tile([128, 4, nq], F32, tag="att")
                    for j in range(4):
                        for dsp in range(2):
                            nc.tensor.matmul(
                                att_ps[:, j, :],
                                lhsT=dbT_sb[:, j, 2 * dsp:2 * dsp + 2, :],
                                rhs=qT_sb[:, 2 * dsp:2 * dsp + 2, :],
                                start=(dsp == 0 and j % 2 == 0),
                                stop=(dsp == 1 and j % 2 == 1),
                                perf_mode=DR,
                            )
                    att_tiles[st] = att_ps

                _load(0)
                _load(1)
                _att(0)
                for st in range(nst):
                    if st + 2 < nst:
                        _load(st + 2)
                    w_sb = wpool.tile([128, 4, nq], FP8, tag="w")
                    nc.scalar.activation(
                        w_sb[:], att_tiles.pop(st)[:], AF.Exp, scale=TAU,
                        bias=ebias_sb[:],
                    )
                    if st + 1 < nst:
                        _att(st + 1)
                    db_sb = db_tiles[st][1]
                    if not no_dma:
                        del db_tiles[st]
                    for jj in range(2):
                        for ds in range(DS):
                            nc.tensor.matmul(
                                zT_ps[:, ds, :],
                                lhsT=db_sb[:, 2 * jj:2 * jj + 2,
                                           ds * 128:(ds + 1) * 128],
                                rhs=w_sb[:, 2 * jj:2 * jj + 2, :],
                                start=(st == 0 and jj == 0 and ds in (0, 2)),
                                stop=(st == nst - 1 and jj == 1 and ds in (1, 3)),
                                perf_mode=DR,
                            )
                    if not no_s:
                        if s_dve:
                            # per-lane partial softmax sums on the DVE
                            # (4 unit-stride adds over the j-subtiles)
                            for j in range(4):
                                nc.vector.tensor_tensor(
                                    s_acc[:], s_acc[:], w_sb[:, j, :], ALU.add
                                )
                        else:
                            for jj in range(2):
                                nc.tensor.matmul(
                                    s_ps[:],
                                    lhsT=ones2_f8[:, :, 0:2],
                                    rhs=w_sb[:, 2 * jj:2 * jj + 2, :],
                                    start=(st == 0 and jj == 0),
                                    stop=(st == nst - 1 and jj == 1),
                                    perf_mode=DR,
                                )

                # ---- softmax normalization of z ----
                s_sb = cpool.tile([1, nq], F32)
                if no_s:
                    nc.vector.memset(s_sb[:], 1.0)
                elif s_dve:
                    # partition-reduce the DVE accumulator with one tiny
                    # fp32 matmul: s[0, q] = sum_p s_acc[p, q]
                    sr_ps = psA.tile([1, nq], F32, tag="att")
                    nc.tensor.matmul(
                        sr_ps[:], lhsT=ones128_f32[:], rhs=s_acc[:],
                        start=True, stop=True,
                    )
                    nc.vector.tensor_copy(s_sb[:], sr_ps[:])
                else:
                    nc.vector.tensor_copy(s_sb[:], s_ps[0:1, :])
                rinv_sb = cpool.tile([1, nq], F32)
                nc.vector.reciprocal(rinv_sb[:], s_sb[:])
                # broadcast 1/s (bf16 path) and S_Z/s (fp8 path) to all
                # 128 partitions
                rb2_ps = psA.tile([128, 2, nq], F32, tag="att")
                nc.tensor.matmul(
                    rb2_ps[:, 0], lhsT=ones1_f32[:], rhs=rinv_sb[:],
                    start=True, stop=False,
                )
                nc.tensor.matmul(
                    rb2_ps[:, 1], lhsT=sz_f32[:], rhs=rinv_sb[:],
                    start=False, stop=True,
                )
                rb2_sb = cpool.tile([128, 2, nq], F32)
                nc.vector.tensor_copy(rb2_sb[:], rb2_ps[:])
                zT8_sb = cpool.tile([128, DS, nq], FP8)
                zTb_sb = cpool.tile([128, DS, nq], BF16)
                for ds in range(DS):
                    nc.vector.tensor_tensor(
                        zT8_sb[:, ds], zT_ps[:, ds], rb2_sb[:, 1], ALU.mult
                    )
                    nc.vector.tensor_tensor(
                        zTb_sb[:, ds], zT_ps[:, ds], rb2_sb[:, 0], ALU.mult
                    )
                # transposes + target-logit dot; with tail_early the PE does
                # the transposes right away and the DVE/ACT chain that
                # follows overlaps the CE matmul stream
                def _tail():
                    z_sb = cpool.tile([128, qt, D], F32)
                    for ds in range(DS):
                        for q in range(qt):
                            tp_ps = psA.tile([128, 128], BF16, tag="att")
                            nc.tensor.transpose(
                                tp_ps[:], zTb_sb[:, ds, q * 128:(q + 1) * 128],
                                ident[:]
                            )
                            nc.vector.tensor_copy(
                                z_sb[:, q, ds * 128:(ds + 1) * 128], tp_ps[:]
                            )
                    tl_sb = cpool.tile([128, qt], F32)
                    prod_sb = cpool.tile([128, D], F32)
                    for q in range(qt):
                        nc.vector.tensor_tensor(
                            prod_sb[:], z_sb[:, q], wt_sb[:, q], ALU.mult
                        )
                        nc.vector.tensor_reduce(
                            tl_sb[:, q:q + 1], prod_sb[:], AX.X, ALU.add
                        )
                    nc.vector.tensor_tensor(tl_sb[:], tl_sb[:], bt_sb[:], ALU.add)
                    return tl_sb

                if tail_early:
                    tl_sb = _tail()

                # ---- classifier + CE ----
                sep_sb = cpool.tile([128, qt, len(cch)], F32)
                if no_ce:
                    nc.vector.memset(sep_sb[:], 1.0)
                for q in range(qt if not no_ce else 0):
                    for cp in range(0, len(cch), 2):
                        # two chunks share one 2-bank psum slot (one bank each)
                        g2_ps = psA.tile([128, 2, 512], F32, tag="att")
                        for k in range(2):
                            if cp + k >= len(cch):
                                break
                            ci = cp + k
                            c0, cw = cch[ci]
                            g_ps = g2_ps[:, k]
                            for dsp in range(2):
                                nc.tensor.matmul(
                                    g_ps[:, :cw],
                                    lhsT=zT8_sb[:, 2 * dsp:2 * dsp + 2,
                                                q * 128:(q + 1) * 128],
                                    rhs=fcw_sb[:, 2 * dsp:2 * dsp + 2,
                                               c0:c0 + cw],
                                    start=(dsp == 0),
                                    stop=(ce_nobias and dsp == 1),
                                    perf_mode=DR,
                                )
                            if not ce_nobias:
                                nc.tensor.matmul(
                                    g_ps[:, :cw],
                                    lhsT=ones1_bf[:],
                                    rhs=fcb_sb[:, c0:c0 + cw],
                                    start=False,
                                    stop=True,
                                )
                            if ce_noexp:
                                if ci == 0:
                                    nc.vector.memset(sep_sb[:, q], 1.0)
                                e_sb = wpool.tile([128, 512], BF16, tag="e")
                                nc.vector.tensor_copy(e_sb[:, :cw], g_ps[:, :cw])
                            else:
                                e_sb = wpool.tile([128, 512], BF16, tag="e")
                                nc.scalar.activation(
                                    e_sb[:, :cw],
                                    g_ps[:, :cw],
                                    AF.Exp,
                                    scale=1.0 / (S_Z * S_W),
                                    accum_out=sep_sb[:, q, ci:ci + 1],
                                )

                if not tail_early:
                    tl_sb = _tail()

                se_sb = cpool.tile([128, qt], F32)
                nc.vector.tensor_reduce(se_sb[:], sep_sb[:], AX.X, ALU.add)
                lse_sb = cpool.tile([128, qt], F32)
                nc.scalar.activation(lse_sb[:], se_sb[:], AF.Ln)
                nll_sb = cpool.tile([128, qt], F32)
                nc.vector.tensor_tensor(nll_sb[:], lse_sb[:], tl_sb[:], ALU.subtract)
                nc.sync.dma_start(out_d[:], nll_sb[:])

    nc.compile()
    return nc


def prep_inputs(q, db_vecs, db_labels, fc_w, fc_b, nst=NST, n_classes=C, nq=NQ,
                n_cores=N_CORES):
    """Host-side sharding / layout prep.  Returns per-core input maps."""
    f8 = NP_FP8
    qt = nq // 128

    # shared (core-independent) layouts
    dbT_h = np.ascontiguousarray(
        db_vecs.reshape(nst, 4, 128, DS, 128).transpose(0, 4, 1, 3, 2)
    ).astype(f8)                                         # [st, p(d_in), j, ds, n']
    db_h = np.ascontiguousarray(
        db_vecs.reshape(nst, 4, 128, D).transpose(0, 2, 1, 3)
    ).astype(f8)                                         # [st, n', j, d]
    fcw_h = np.ascontiguousarray(
        (fc_w.T * S_W).reshape(DS, 128, n_classes).transpose(1, 0, 2)
    ).astype(f8)                                         # [p(d_in), ds, c]
    fcb_h = (fc_b * (S_Z * S_W)).reshape(1, n_classes).astype(ml_dtypes.bfloat16)

    labels = np.asarray(db_labels).reshape(-1)
    in_maps = []
    for core in range(n_cores):
        q_c = q[core * nq:(core + 1) * nq]               # [nq, D]
        qT_h = np.ascontiguousarray(
            q_c.T.reshape(DS, 128, nq).transpose(1, 0, 2)
        ).astype(f8)                                     # [p(d_in), ds, q]
        lab = labels[core * nq:(core + 1) * nq].astype(np.int64)
        wt_h = np.ascontiguousarray(
            fc_w[lab].reshape(qt, 128, D).transpose(1, 0, 2)
        ).astype(np.float32)                             # [p(q_in), qt, d]
        bt_h = np.ascontiguousarray(
            fc_b[lab].reshape(qt, 128).T
        ).astype(np.float32)                             # [p(q_in), qt]
        in_maps.append({
            "qT": qT_h, "dbT": dbT_h, "db": db_h, "fcw": fcw_h,
            "fcb": fcb_h, "wt": wt_h, "bt": bt_h,
        })
    return in_maps


def kernel(q, db_vecs, db_labels, fc_w, fc_b, _return_results=False, **run_kwargs):
    q = np.asarray(q, np.float32)
    db_vecs = np.asarray(db_vecs, np.float32)
    fc_w = np.asarray(fc_w, np.float32)
    fc_b = np.asarray(fc_b, np.float32)

    zero_bias = not np.any(fc_b)
    key = ("nc", zero_bias)
    if key not in _CACHE:
        _CACHE[key] = build_nc(ce_nobias=zero_bias)
    nc = _CACHE[key]

    in_maps = prep_inputs(q, db_vecs, db_labels, fc_w, fc_b)
    res = run_bass_kernel_spmd(nc, in_maps, core_ids=list(range(N_CORES)),
                               **run_kwargs)
    nlls = [r["nll"].T.reshape(-1) for r in res.results]   # [nq] per core
    out = np.float32(np.mean(np.concatenate(nlls)))
    if _return_results:
        return out, res
    return out
